# revision 13
# baseline (speedup 1.0000x reference)
"""NeuralControlCritic Trainium2 kernel — 8-core SPMD, batch-data-parallel.

Network: h = relu(x@W1+b1)@W2+b2 -> reshape (B,8,1024) -> mamba2 x2 -> relu(h@Wo1+bo1)@Wo2+bo2
All activations on-chip are kept feature/channel-major: [features->partitions, (b,s)->free],
token column order col = b*SEQ + s.

Mamba scan uses the SSD O(L^2) form:
  Y[b,t,h,p] = sum_s W[b,h,t,s] * X[b,s,h,p],
  W = exp(cumla_t - (cumla_s - ln dt_s)) * G[b,t,s]  (+ D[h] on t==s diag)
  G[b,t,s] = sum_d C[b,t,d]*B[b,s,d]   (PE gram + diag-mask + grouped reduce)
The d_inner channel axis is host-permuted from (h,p) to (p,h) order so the scan's
per-(h,b) weight expands to channel tiles by pure partition-block replication.
"""
import numpy as np
import ml_dtypes
from contextlib import ExitStack

import concourse.bass as bass
import concourse.bacc as bacc
import concourse.mybir as mybir
import concourse.tile as tile
from concourse import library_config
from concourse.bass_utils import run_bass_kernel_spmd
from concourse._compat import with_exitstack

F32 = mybir.dt.float32
BF16 = mybir.dt.bfloat16
AF = mybir.ActivationFunctionType
ALU = mybir.AluOpType
AX = mybir.AxisListType
BF16NP = ml_dtypes.bfloat16

HSS = 1024
SEQ = 8
HID = 1024
REWARD = 2
BATCH = 256
DIN = 2048
HD = 64
NH = 32
DS = 128
DCONV = 4
CONVD = DIN + 2 * DS            # 2304
DPROJ = 2 * DIN + 2 * DS + NH   # 4384
NCORES = 8
BC = BATCH // NCORES            # 32 batch rows per core
NT = BC * SEQ                   # 256 tokens per core; col = b*SEQ + s

# channel permutation (h,p) -> (p,h): NEWPERM[p*NH+h] = h*HD+p
_PERM = np.arange(DIN).reshape(NH, HD).T.reshape(-1)  # new c = p*32+h maps to old h*64+p


def _perm_cols_z_x(in_proj):
    """Permute z and x column blocks of in_proj to (p,h) channel order."""
    w = np.array(in_proj, dtype=np.float32, copy=True)
    w[:, 0:DIN] = w[:, 0:DIN][:, _PERM]
    w[:, DIN:2 * DIN] = w[:, DIN:2 * DIN][:, _PERM]
    return w


def _host_prep(inputs):
    """Build the per-core input maps (all host-side transforms are free)."""
    f32 = lambda a: np.ascontiguousarray(np.asarray(a, dtype=np.float32))
    bf = lambda a: np.ascontiguousarray(np.asarray(a, dtype=np.float32).astype(BF16NP))

    common = {}
    common["w_in1"] = bf(inputs["w_in1"])
    common["b_in1"] = np.ascontiguousarray(f32(inputs["b_in1"]).reshape(64, 128).T)
    common["w_in2"] = bf(inputs["w_in2"])
    common["b_in2"] = np.ascontiguousarray(f32(inputs["b_in2"]).reshape(64, 128).T)
    for li in (1, 2):
        p = f"m{li}_"
        common[p + "inproj"] = bf(_perm_cols_z_x(inputs[p + "in_proj"]))
        cw = f32(inputs[p + "conv_w"])
        cb = f32(inputs[p + "conv_b"])
        cw = np.concatenate([cw[0:DIN][_PERM], cw[DIN:]], axis=0)
        cb = np.concatenate([cb[0:DIN][_PERM], cb[DIN:]], axis=0)
        common[p + "convw"] = np.ascontiguousarray(
            cw.reshape(18, 128, DCONV).transpose(1, 0, 2).reshape(128, 18 * DCONV))
        common[p + "convb"] = np.ascontiguousarray(
            cb.reshape(18, 128).T)                                # [128, 18] f32
        common[p + "dtb"] = f32(inputs[p + "dt_bias"]).reshape(NH, 1)
        common[p + "aneg"] = (-np.exp(f32(inputs[p + "A_log"]))).reshape(NH, 1)
        common[p + "dvec"] = f32(inputs[p + "D"]).reshape(NH, 1)
        common[p + "normw"] = np.ascontiguousarray(
            f32(inputs[p + "norm_w"])[_PERM].reshape(16, 128).T)  # [128, 16] f32
        common[p + "outproj"] = bf(np.array(inputs[p + "out_proj"])[_PERM, :])
    common["w_out1"] = bf(inputs["w_out1"])
    common["b_out1"] = np.ascontiguousarray(f32(inputs["b_out1"]).reshape(8, 128).T)
    common["w_out2"] = bf(inputs["w_out2"])
    common["b_out2"] = f32(inputs["b_out2"]).reshape(REWARD, 1)

    x = np.asarray(inputs["x"], dtype=np.float32)
    in_maps = []
    for c in range(NCORES):
        m = dict(common)
        m["xT"] = np.ascontiguousarray(x[c * BC:(c + 1) * BC, :].T.astype(BF16NP))
        in_maps.append(m)
    return in_maps


def _masks_np():
    """G-gram diag masks with causality baked in: mask_h[(b16,t8),(b',s)]=(b'==b+16h)&(t>=s)."""
    masks = []
    for half in (0, 1):
        m = np.zeros((128, 256), np.float32)
        for r in range(128):
            b, t = r // 8 + 16 * half, r % 8
            for c in range(256):
                bp, s = c // 8, c % 8
                if bp == b and t >= s:
                    m[r, c] = 1.0
        masks.append(m)
    return masks


def _r(ap, spec, **kw):
    return ap.rearrange(spec, **kw)


@with_exitstack
def emit(ctx: ExitStack, tc: "tile.TileContext", ins: dict, out_ap):
    nc = tc.nc

    cpool = ctx.enter_context(tc.tile_pool(name="consts", bufs=1))
    apool = ctx.enter_context(tc.tile_pool(name="acts", bufs=1))
    wbig = ctx.enter_context(tc.tile_pool(name="wbig", bufs=2))     # [128,8192] streams
    kcol = ctx.enter_context(tc.tile_pool(name="kcol", bufs=4))     # [128,<=2048] weight cols
    rot = ctx.enter_context(tc.tile_pool(name="rot", bufs=3))       # rotating temps

    # ---- constants ----
    m0, m1 = _masks_np()
    mask_t = [nc.inline_tensor(m0, name="gmask0"), nc.inline_tensor(m1, name="gmask1")]
    masks = []
    for i in (0, 1):
        mt = cpool.tile([128, 256], F32, tag=f"mask{i}")
        nc.sync.dma_start(mt[:], mask_t[i][:, :])
        masks.append(mt)
    ones32f = cpool.tile([1, 32], F32, tag="ones32f")
    nc.vector.memset(ones32f[:], 1.0)
    ones128f = cpool.tile([1, 128], F32, tag="ones128f")
    nc.vector.memset(ones128f[:], 1.0)
    onesb = cpool.tile([128, 1], BF16, tag="onesb")
    nc.vector.memset(onesb[:], 1.0)

    # ---- stage 1: mlp_in ----
    # xT sbuf: [128, (kt8, b32)]
    xT = apool.tile([128, SEQ * BC], BF16, tag="xT")
    nc.sync.dma_start(
        xT[:], _r(ins["xT"][:, :], "(kt p) b -> p kt b", p=128))
    b_in1 = cpool.tile([128, 64], F32, tag="b_in1")
    nc.sync.dma_start(b_in1[:], ins["b_in1"][:, :])
    b_in2 = cpool.tile([128, 64], F32, tag="b_in2")
    nc.sync.dma_start(b_in2[:], ins["b_in2"][:, :])

    # GEMM1: h1T[f,b] = W1[:,f].T @ xT ; 64 m-tiles, 8 k-tiles
    pbig_cm = tc.tile_pool(name="pbig", bufs=1, space="PSUM")
    pbig = pbig_cm.__enter__()
    ps1 = pbig.tile([128, 64 * BC], F32, tag="psbig")
    for mt in range(64):
        w1c = kcol.tile([128, 8 * 128], BF16, tag="kcol")
        nc.sync.dma_start(
            w1c[:], _r(ins["w_in1"][:, mt * 128:(mt + 1) * 128],
                       "(kt p) j -> p kt j", p=128))
        sl = ps1[:, mt * BC:(mt + 1) * BC]
        for kt in range(8):
            nc.tensor.matmul(sl, w1c[:, kt * 128:(kt + 1) * 128],
                             xT[:, kt * BC:(kt + 1) * BC],
                             start=(kt == 0), stop=(kt == 7))
    nc.vector.tensor_add(
        _r(ps1[:, :], "p (mt b) -> p mt b", mt=64),
        _r(ps1[:, :], "p (mt b) -> p mt b", mt=64),
        b_in1[:, :].unsqueeze(2).broadcast_to((128, 64, BC)))
    h1T = apool.tile([128, 64 * BC], BF16, tag="h1T")
    nc.scalar.activation(h1T[:], ps1[:], AF.Relu)

    # GEMM2: h2T = W2[:,f].T @ h1 ; psum free order (mt=(s,j), b) -> h2cm free (j,b,s)
    ps2 = pbig.tile([128, 64 * BC], F32, tag="psbig")
    for mt in range(64):
        w2c = wbig.tile([128, 64 * 128], BF16, tag="wbig")
        src = _r(ins["w_in2"][:, mt * 128:(mt + 1) * 128], "(kt p) j -> p kt j", p=128)
        for ch in range(4):
            nc.sync.dma_start(w2c[:, ch * 2048:(ch + 1) * 2048], src[:, ch * 16:(ch + 1) * 16, :])
        sl = ps2[:, mt * BC:(mt + 1) * BC]
        for kt in range(64):
            nc.tensor.matmul(sl, w2c[:, kt * 128:(kt + 1) * 128],
                             h1T[:, kt * BC:(kt + 1) * BC],
                             start=(kt == 0), stop=(kt == 63))
    nc.vector.tensor_add(
        _r(ps2[:, :], "p (mt b) -> p mt b", mt=64),
        _r(ps2[:, :], "p (mt b) -> p mt b", mt=64),
        b_in2[:, :].unsqueeze(2).broadcast_to((128, 64, BC)))
    # h2cm[p, (j, b, s)] <- ps2[p, (s, j, b)]
    h2cm = apool.tile([128, 8 * NT], BF16, tag="h2cm")
    nc.vector.tensor_copy(
        _r(h2cm[:, :], "p (j b s) -> p j b s", j=8, b=BC),
        _r(ps2[:, :], "p (s j b) -> p s j b", s=8, j=8).transpose([0, 2, 3, 1]))

    pbig_cm.__exit__(None, None, None)
    hin = h2cm
    for li in (1, 2):
        hin = mamba_layer(tc, ctx, ins, f"m{li}_", hin,
                          cpool, apool, kcol, rot,
                          masks, ones32f, ones128f, onesb)

    # ---- stage 3: mlp_out ----
    b_out1 = cpool.tile([128, 8], F32, tag="b_out1")
    nc.sync.dma_start(b_out1[:], ins["b_out1"][:, :])
    b_out2 = cpool.tile([REWARD, 1], F32, tag="b_out2")
    nc.sync.dma_start(b_out2[:], ins["b_out2"][:, :])
    h4v = _r(hin[:, :], "p (j b s) -> p j b s", j=8, b=BC)

    pout_cm = tc.tile_pool(name="pout", bufs=1, space="PSUM")
    pout = pout_cm.__enter__()
    ps3 = pout.tile([128, 8 * BC], F32, tag="ps3")
    for mt in range(8):
        woc = wbig.tile([128, 64 * 128], BF16, tag="wbig")
        src = _r(ins["w_out1"][:, mt * 128:(mt + 1) * 128], "(kt p) j -> p kt j", p=128)
        for ch in range(4):
            nc.sync.dma_start(woc[:, ch * 2048:(ch + 1) * 2048], src[:, ch * 16:(ch + 1) * 16, :])
        sl = ps3[:, mt * BC:(mt + 1) * BC]
        for kt in range(64):
            s, j = kt // 8, kt % 8
            nc.tensor.matmul(sl, woc[:, kt * 128:(kt + 1) * 128],
                             h4v[:, j, :, s], start=(kt == 0), stop=(kt == 63))
    nc.vector.tensor_add(
        _r(ps3[:, :], "p (mt b) -> p mt b", mt=8),
        _r(ps3[:, :], "p (mt b) -> p mt b", mt=8),
        b_out1[:, :].unsqueeze(2).broadcast_to((128, 8, BC)))
    h5 = apool.tile([128, 8 * BC], BF16, tag="h5")
    nc.scalar.activation(h5[:], ps3[:], AF.Relu)

    wo2 = cpool.tile([128, 8 * REWARD], BF16, tag="wo2")
    nc.sync.dma_start(wo2[:], _r(ins["w_out2"][:, :], "(kt p) r -> p kt r", p=128))
    ps4 = pout.tile([REWARD, BC], F32, tag="ps4")
    for kt in range(8):
        nc.tensor.matmul(ps4[:], wo2[:, kt * REWARD:(kt + 1) * REWARD],
                         h5[:, kt * BC:(kt + 1) * BC], start=(kt == 0), stop=(kt == 7))
    outT = apool.tile([REWARD, BC], F32, tag="outT")
    nc.vector.tensor_scalar_add(outT[:], ps4[:], b_out2[:, 0:1])
    nc.sync.dma_start(out_ap[:, :], outT[:])
    pout_cm.__exit__(None, None, None)


def mamba_layer(tc, ctx, ins, pfx, hin, cpool, apool, kcol, rot,
                masks, ones32f, ones128f, onesb):
    nc = tc.nc
    li = pfx  # tag prefix
    pp_cm = tc.tile_pool(name=pfx + "psum", bufs=1, space="PSUM")
    pp = pp_cm.__enter__()

    convw = cpool.tile([128, 18 * DCONV], F32, tag=li + "convw")
    nc.sync.dma_start(convw[:], ins[pfx + "convw"][:, :])
    convb = cpool.tile([128, 18], F32, tag=li + "convb")
    nc.sync.dma_start(convb[:], ins[pfx + "convb"][:, :])
    dtb = cpool.tile([NH, 1], F32, tag=li + "dtb")
    nc.sync.dma_start(dtb[:], ins[pfx + "dtb"][:, :])
    aneg = cpool.tile([NH, 1], F32, tag=li + "aneg")
    nc.sync.dma_start(aneg[:], ins[pfx + "aneg"][:, :])
    dvec = cpool.tile([NH, 1], F32, tag=li + "dvec")
    nc.sync.dma_start(dvec[:], ins[pfx + "dvec"][:, :])
    normw = cpool.tile([128, 16], F32, tag=li + "normw")
    nc.sync.dma_start(normw[:], ins[pfx + "normw"][:, :])

    zbf = apool.tile([128, 16 * NT], BF16, tag="mb_zbf")
    xact = apool.tile([128, 18 * NT], BF16, tag="mb_xact")
    dt_raw = apool.tile([NH, NT], F32, tag="mb_dt_raw")

    # in_proj GEMM: m-tile order B,C,dt then x tiles then z tiles
    order = [32, 33, 34] + list(range(16, 32)) + list(range(16))
    for mt in order:
        mcols = 128 if mt != 34 else NH
        wc = kcol.tile([128, 8 * 128], BF16, tag="kcol")
        nc.sync.dma_start(
            wc[:, 0:8 * mcols],
            _r(ins[pfx + "inproj"][:, mt * 128:mt * 128 + mcols],
               "(kt p) j -> p kt j", p=128))
        ps = pp.tile([128, NT], F32, tag="psin" + str(mt % 2))
        pss = ps[0:mcols, :]
        for kt in range(8):
            nc.tensor.matmul(pss, wc[:, kt * mcols:(kt + 1) * mcols],
                             hin[:, kt * NT:(kt + 1) * NT],
                             start=(kt == 0), stop=(kt == 7))
        if mt < 16:
            nc.scalar.copy(zbf[:, mt * NT:(mt + 1) * NT], ps[:])
        elif mt == 34:
            nc.scalar.copy(dt_raw[:], pss)
        else:
            i = mt - 16
            # conv + silu for this channel tile
            xbc = rot.tile([128, NT], F32, tag="xbc")
            nc.scalar.copy(xbc[:], ps[:])
            cw = convw[:, i * DCONV:(i + 1) * DCONV]
            cb = convb[:, i:i + 1]
            conv = rot.tile([128, NT], F32, tag="conv")
            xv = _r(xbc[:, :], "p (b s) -> p b s", b=BC)
            cv = _r(conv[:, :], "p (b s) -> p b s", b=BC)
            nc.vector.tensor_scalar(conv[:], xbc[:], cw[:, 3:4], cb,
                                    ALU.mult, ALU.add)
            for k in (2, 1, 0):
                d = 3 - k
                nc.vector.scalar_tensor_tensor(
                    cv[:, :, d:8], xv[:, :, 0:8 - d], cw[:, k:k + 1],
                    cv[:, :, d:8], ALU.mult, ALU.add)
            sg = rot.tile([128, NT], F32, tag="sg")
            nc.scalar.activation(sg[:], conv[:], AF.Sigmoid)
            nc.vector.tensor_mul(xact[:, i * NT:(i + 1) * NT], conv[:], sg[:])

    # ---- dt path ----
    dt = apool.tile([NH, NT], F32, tag="mb_dt")
    nc.scalar.activation(dt[:], dt_raw[:], AF.Exp, bias=dtb[:, 0:1])
    nc.scalar.activation(dt[:], dt[:], AF.Ln, bias=1.0)       # softplus
    lndt = apool.tile([NH, NT], F32, tag="mb_lndt")
    nc.scalar.activation(lndt[:], dt[:], AF.Ln)
    la = apool.tile([NH, NT], F32, tag="mb_la")
    nc.vector.tensor_scalar_mul(la[:], dt[:], aneg[:, 0:1])
    # cumsum along s (within b-groups of 8)
    lav = _r(la[:, :], "p (b s) -> p b s", b=BC)
    t1 = apool.tile([NH, NT], F32, tag="mb_cum1")
    t1v = _r(t1[:, :], "p (b s) -> p b s", b=BC)
    nc.vector.tensor_copy(t1v[:, :, 0:1], lav[:, :, 0:1])
    nc.vector.tensor_add(t1v[:, :, 1:8], lav[:, :, 1:8], lav[:, :, 0:7])
    t2 = apool.tile([NH, NT], F32, tag="mb_cum2")
    t2v = _r(t2[:, :], "p (b s) -> p b s", b=BC)
    nc.vector.tensor_copy(t2v[:, :, 0:2], t1v[:, :, 0:2])
    nc.vector.tensor_add(t2v[:, :, 2:8], t1v[:, :, 2:8], t1v[:, :, 0:6])
    cumla = apool.tile([NH, NT], F32, tag="mb_cumla")
    cv_ = _r(cumla[:, :], "p (b s) -> p b s", b=BC)
    nc.vector.tensor_copy(cv_[:, :, 0:4], t2v[:, :, 0:4])
    nc.vector.tensor_add(cv_[:, :, 4:8], t2v[:, :, 4:8], t2v[:, :, 0:4])
    cumlaA = apool.tile([NH, NT], F32, tag="mb_cumlaA")
    nc.vector.tensor_sub(cumlaA[:], cumla[:], lndt[:])

    # ---- G via PE gram + masked grouped reduce ----
    xB = xact[:, 16 * NT:17 * NT]
    xC = xact[:, 17 * NT:18 * NT]
    gh = []
    for half in (0, 1):
        psg = pp.tile([128, NT], F32, tag="psg")
        nc.tensor.matmul(psg[:], xC[:, half * 128:(half + 1) * 128], xB,
                         start=True, stop=True)
        mg = rot.tile([128, NT], F32, tag="mg")
        nc.vector.tensor_mul(mg[:], psg[:], masks[half][:])
        g = rot.tile([128, 8], F32, tag="ghalf")
        nc.vector.tensor_reduce(g[:], _r(mg[:, :], "p (b s) -> p s b", b=BC),
                                AX.X, ALU.add)
        gh.append(g)
    # flip G[(b,t),s] -> row [1,(b,t,s)] via SBUF->SBUF DMA flatten
    grow = apool.tile([1, 2048], F32, tag="mb_grow")
    for half in (0, 1):
        nc.sync.dma_start(grow[0:1, half * 1024:(half + 1) * 1024], gh[half][:, :])
    # replicate to 32 partitions via PE rank-1: psum[32,512] = ones32f.T @ grow-slice
    gb = apool.tile([NH, 2048], F32, tag="mb_gb")
    for q in range(4):
        pb = pp.tile([NH, 512], F32, tag="pbc")
        nc.tensor.matmul(pb[:], ones32f[0:1, :], grow[0:1, q * 512:(q + 1) * 512],
                         start=True, stop=True)
        nc.scalar.copy(gb[:, q * 512:(q + 1) * 512], pb[:])

    # ---- E = exp(cumla_t - cumlaA_s), W = E*G (+D diag), replicate -> Wexp ----
    w32 = apool.tile([NH, 2048], F32, tag="mb_w32")
    cb_t = _r(cumla[:, :], "p (b s) -> p b s", b=BC).unsqueeze(3).broadcast_to((NH, BC, 8, 8))
    ca_s = _r(cumlaA[:, :], "p (b s) -> p b s", b=BC).unsqueeze(2).broadcast_to((NH, BC, 8, 8))
    nc.vector.tensor_sub(_r(w32[:, :], "p (b t s) -> p b t s", b=BC, t=8), cb_t, ca_s)
    nc.vector.tensor_scalar_min(w32[:], w32[:], 30.0)  # padded t<s pairs would overflow exp
    nc.scalar.activation(w32[:], w32[:], AF.Exp)
    nc.vector.tensor_mul(w32[:], w32[:], gb[:])
    w32v = _r(w32[:, :], "p (b t s) -> p b t s", b=BC, t=8)
    for t in range(8):
        nc.vector.tensor_scalar_add(w32v[:, :, t:t + 1, t:t + 1],
                                    w32v[:, :, t:t + 1, t:t + 1], dvec[:, 0:1])
    wexp = apool.tile([128, 2048], BF16, tag="mb_wexp")
    nc.vector.tensor_copy(wexp[0:32, :], w32[:])
    nc.vector.tensor_copy(wexp[32:64, :], wexp[0:32, :])
    nc.vector.tensor_copy(wexp[64:128, :], wexp[0:64, :])

    # ---- scan: per x channel tile: Y = reduce_s( Xbcast * Wexp ) ----
    y2 = apool.tile([128, 16 * NT], F32, tag="mb_y2")
    psum_ss = pp.tile([1, NT], F32, tag="pss")
    for i in range(16):
        xi = xact[:, i * NT:(i + 1) * NT]
        xbc_ap = _r(xi, "p (b s) -> p b s", b=BC).unsqueeze(2).broadcast_to((128, BC, 8, 8))
        tmp = rot.tile([128, 2048], BF16, tag="scantmp")
        nc.vector.tensor_mul(_r(tmp[:, :], "p (b t s) -> p b t s", b=BC, t=8),
                             xbc_ap, _r(wexp[:, :], "p (b t s) -> p b t s", b=BC, t=8))
        yi = rot.tile([128, NT], F32, tag="yi")
        nc.vector.tensor_reduce(yi[:], _r(tmp[:, :], "p (b t s) -> p b t s", b=BC, t=8),
                                AX.X, ALU.add)
        # gating: y * z * sigmoid(z)
        zi = zbf[:, i * NT:(i + 1) * NT]
        sgz = rot.tile([128, NT], BF16, tag="sgz")
        nc.scalar.activation(sgz[:], zi, AF.Sigmoid)
        y2i = y2[:, i * NT:(i + 1) * NT]
        nc.vector.tensor_mul(yi[:], yi[:], zi)
        nc.vector.tensor_mul(y2i, yi[:], sgz[:])
        ybf = rot.tile([128, NT], BF16, tag="ybf")
        nc.vector.tensor_mul(ybf[:], y2i, y2i)
        nc.tensor.matmul(psum_ss[:], onesb[:, :], ybf[:],
                         start=(i == 0), stop=(i == 15))
    # Note: ybf = y2^2 cast to bf16; summed over channels via ones-matmul.
    epsb = cpool.tile([1, 1], F32, tag=li + "epsb")
    nc.vector.memset(epsb[:], 1e-5)
    rsq = apool.tile([1, NT], F32, tag="mb_rsq")
    nc.scalar.activation(rsq[:], psum_ss[:], AF.Sqrt, bias=epsb[0:1, 0:1], scale=1.0 / DIN)
    nc.vector.reciprocal(rsq[:], rsq[:])
    rsqb = apool.tile([128, NT], F32, tag="mb_rsqb")
    prs = pp.tile([128, NT], F32, tag="psg")
    nc.tensor.matmul(prs[:], ones128f[0:1, :], rsq[0:1, :], start=True, stop=True)
    nc.scalar.copy(rsqb[:], prs[:])
    ynorm = apool.tile([128, 16 * NT], BF16, tag="mb_ynorm")
    for i in range(16):
        nc.vector.scalar_tensor_tensor(
            ynorm[:, i * NT:(i + 1) * NT], y2[:, i * NT:(i + 1) * NT],
            normw[:, i:i + 1], rsqb[:], ALU.mult, ALU.mult)

    # ---- out_proj ----
    hout = apool.tile([128, 8 * NT], BF16, tag=li + "hout")
    for mt in range(8):
        wc = kcol.tile([128, 16 * 128], BF16, tag="kcolw")
        nc.sync.dma_start(
            wc[:], _r(ins[pfx + "outproj"][:, mt * 128:(mt + 1) * 128],
                      "(kt p) j -> p kt j", p=128))
        ps = pp.tile([128, NT], F32, tag="psout" + str(mt % 2))
        for kt in range(16):
            nc.tensor.matmul(ps[:], wc[:, kt * 128:(kt + 1) * 128],
                             ynorm[:, kt * NT:(kt + 1) * NT],
                             start=(kt == 0), stop=(kt == 15))
        nc.scalar.copy(hout[:, mt * NT:(mt + 1) * NT], ps[:])
    pp_cm.__exit__(None, None, None)
    return hout


_CACHE = {}


def _build():
    if "nc" in _CACHE:
        return _CACHE["nc"], _CACHE["innames"], _CACHE["outname"]
    in_specs = {
        "xT": ([HSS, BC], BF16),
        "w_in1": ([HSS, HSS * SEQ], BF16), "b_in1": ([128, 64], F32),
        "w_in2": ([HSS * SEQ, HSS * SEQ], BF16), "b_in2": ([128, 64], F32),
        "w_out1": ([HID * SEQ, HID], BF16), "b_out1": ([128, 8], F32),
        "w_out2": ([HID, REWARD], BF16), "b_out2": ([REWARD, 1], F32),
    }
    for li in (1, 2):
        p = f"m{li}_"
        in_specs.update({
            p + "inproj": ([HSS, DPROJ], BF16),
            p + "convw": ([128, 18 * DCONV], F32),
            p + "convb": ([128, 18], F32),
            p + "dtb": ([NH, 1], F32),
            p + "aneg": ([NH, 1], F32),
            p + "dvec": ([NH, 1], F32),
            p + "normw": ([128, 16], F32),
            p + "outproj": ([DIN, HID], BF16),
        })
    nc = bacc.Bacc("TRN2", target_bir_lowering=False, debug=False,
                   enable_asserts=True, num_devices=NCORES)
    ins = {}
    for name, (shape, dt) in in_specs.items():
        ins[name] = nc.dram_tensor(name, shape, dt, kind="ExternalInput").ap()
    out_ap = nc.dram_tensor("out", [REWARD, BC], F32, kind="ExternalOutput").ap()
    with tile.TileContext(nc) as tc:
        emit(tc, ins, out_ap)
    nc.compile()
    _CACHE["nc"] = nc
    _CACHE["innames"] = list(in_specs.keys())
    _CACHE["outname"] = "out"
    return nc, _CACHE["innames"], "out"


def kernel(**inputs) -> np.ndarray:
    nc, innames, outname = _build()
    in_maps = _host_prep(inputs)
    res = run_bass_kernel_spmd(nc, in_maps, core_ids=list(range(NCORES)))
    out = np.zeros((BATCH, REWARD), np.float32)
    for c in range(NCORES):
        out[c * BC:(c + 1) * BC, :] = np.asarray(res.results[c][outname]).T
    return out


if __name__ == "__main__":
    rng = np.random.default_rng(0)
    fake = {"x": rng.standard_normal((BATCH, HSS), dtype=np.float32)}
    print("smoke build only")
    _build()
    print("build ok")


# revision 14
# speedup vs baseline: 1.4746x; 1.4746x over previous
"""NeuralControlCritic Trainium2 kernel — 8-core SPMD, batch-data-parallel.

Network: h = relu(x@W1+b1)@W2+b2 -> reshape (B,8,1024) -> mamba2 x2 -> relu(h@Wo1+bo1)@Wo2+bo2
All activations on-chip are kept feature/channel-major: [features->partitions, (b,s)->free],
token column order col = b*SEQ + s.

Mamba scan uses the SSD O(L^2) form:
  Y[b,t,h,p] = sum_s W[b,h,t,s] * X[b,s,h,p],
  W = exp(cumla_t - (cumla_s - ln dt_s)) * G[b,t,s]  (+ D[h] on t==s diag)
  G[b,t,s] = sum_d C[b,t,d]*B[b,s,d]   (PE gram + diag-mask + grouped reduce)
The d_inner channel axis is host-permuted from (h,p) to (p,h) order so the scan's
per-(h,b) weight expands to channel tiles by pure partition-block replication.
"""
import numpy as np
import ml_dtypes
from contextlib import ExitStack

import concourse.bass as bass
import concourse.bacc as bacc
import concourse.mybir as mybir
import concourse.tile as tile
from concourse import library_config
from concourse.bass_utils import run_bass_kernel_spmd
from concourse._compat import with_exitstack

F32 = mybir.dt.float32
BF16 = mybir.dt.bfloat16
AF = mybir.ActivationFunctionType
ALU = mybir.AluOpType
AX = mybir.AxisListType
BF16NP = ml_dtypes.bfloat16

HSS = 1024
SEQ = 8
HID = 1024
REWARD = 2
BATCH = 256
DIN = 2048
HD = 64
NH = 32
DS = 128
DCONV = 4
CONVD = DIN + 2 * DS            # 2304
DPROJ = 2 * DIN + 2 * DS + NH   # 4384
NCORES = 8
BC = BATCH // NCORES            # 32 batch rows per core
NT = BC * SEQ                   # 256 tokens per core; col = b*SEQ + s

# channel permutation (h,p) -> (p,h): NEWPERM[p*NH+h] = h*HD+p
_PERM = np.arange(DIN).reshape(NH, HD).T.reshape(-1)  # new c = p*32+h maps to old h*64+p


def _perm_cols_z_x(in_proj):
    """Permute z and x column blocks of in_proj to (p,h) channel order."""
    w = np.array(in_proj, dtype=np.float32, copy=True)
    w[:, 0:DIN] = w[:, 0:DIN][:, _PERM]
    w[:, DIN:2 * DIN] = w[:, DIN:2 * DIN][:, _PERM]
    return w


def _host_prep(inputs):
    """Build the per-core input maps (all host-side transforms are free)."""
    f32 = lambda a: np.ascontiguousarray(np.asarray(a, dtype=np.float32))
    bf = lambda a: np.ascontiguousarray(np.asarray(a, dtype=np.float32).astype(BF16NP))

    def wtiles(w, kt, mt):
        # (K, M) -> [mt, 128(p), kt*128(j-major? no: kt blocks of j)] with
        # out[m, p, k*128:(k+1)*128] = w[k*128+p, m*128:(m+1)*128]
        K, M = w.shape
        a = np.asarray(w, dtype=np.float32).reshape(kt, 128, mt, M // mt)
        a = a.transpose(2, 1, 0, 3).reshape(mt, 128, kt * (M // mt))
        return np.ascontiguousarray(a.astype(BF16NP))

    common = {}
    common["w_in1"] = wtiles(inputs["w_in1"], 8, 64)
    common["b_in1"] = np.ascontiguousarray(f32(inputs["b_in1"]).reshape(64, 128).T)
    common["w_in2"] = wtiles(inputs["w_in2"], 64, 64)
    common["b_in2"] = np.ascontiguousarray(f32(inputs["b_in2"]).reshape(64, 128).T)
    for li in (1, 2):
        p = f"m{li}_"
        ipp = _perm_cols_z_x(inputs[p + "in_proj"])
        ipp = np.concatenate([ipp, np.zeros((HSS, 4480 - DPROJ), np.float32)], axis=1)
        common[p + "inproj"] = wtiles(ipp, 8, 35)
        cw = f32(inputs[p + "conv_w"])
        cb = f32(inputs[p + "conv_b"])
        cw = np.concatenate([cw[0:DIN][_PERM], cw[DIN:]], axis=0)
        cb = np.concatenate([cb[0:DIN][_PERM], cb[DIN:]], axis=0)
        common[p + "convw"] = np.ascontiguousarray(
            cw.reshape(18, 128, DCONV).transpose(1, 0, 2).reshape(128, 18 * DCONV))
        common[p + "convb"] = np.ascontiguousarray(
            cb.reshape(18, 128).T)                                # [128, 18] f32
        common[p + "dtb"] = f32(inputs[p + "dt_bias"]).reshape(NH, 1)
        common[p + "aneg"] = (-np.exp(f32(inputs[p + "A_log"]))).reshape(NH, 1)
        common[p + "dvec"] = f32(inputs[p + "D"]).reshape(NH, 1)
        common[p + "normw"] = np.ascontiguousarray(
            f32(inputs[p + "norm_w"])[_PERM].reshape(16, 128).T)  # [128, 16] f32
        common[p + "outproj"] = wtiles(np.array(inputs[p + "out_proj"])[_PERM, :], 16, 8)
    common["w_out1"] = wtiles(inputs["w_out1"], 64, 8)
    common["b_out1"] = np.ascontiguousarray(f32(inputs["b_out1"]).reshape(8, 128).T)
    common["w_out2"] = np.ascontiguousarray(
        f32(inputs["w_out2"]).reshape(8, 128, REWARD).transpose(1, 0, 2)
        .reshape(128, 8 * REWARD).astype(BF16NP))
    common["b_out2"] = f32(inputs["b_out2"]).reshape(REWARD, 1)

    x = np.asarray(inputs["x"], dtype=np.float32)
    in_maps = []
    for c in range(NCORES):
        m = dict(common)
        xt = x[c * BC:(c + 1) * BC, :].T.reshape(8, 128, BC).transpose(1, 0, 2)
        m["xT"] = np.ascontiguousarray(xt.reshape(128, SEQ * BC).astype(BF16NP))
        in_maps.append(m)
    return in_maps


def _masks_np():
    """G-gram diag masks with causality baked in: mask_h[(b16,t8),(b',s)]=(b'==b+16h)&(t>=s)."""
    masks = []
    for half in (0, 1):
        m = np.zeros((128, 256), np.float32)
        for r in range(128):
            b, t = r // 8 + 16 * half, r % 8
            for c in range(256):
                bp, s = c // 8, c % 8
                if bp == b and t >= s:
                    m[r, c] = 1.0
        masks.append(m)
    return masks


def _r(ap, spec, **kw):
    return ap.rearrange(spec, **kw)


@with_exitstack
def emit(ctx: ExitStack, tc: "tile.TileContext", ins: dict, out_ap):
    nc = tc.nc

    cpool = ctx.enter_context(tc.tile_pool(name="consts", bufs=1))
    apool = ctx.enter_context(tc.tile_pool(name="acts", bufs=1))
    wbig = ctx.enter_context(tc.tile_pool(name="wbig", bufs=2))     # [128,8192] streams
    kcol = ctx.enter_context(tc.tile_pool(name="kcol", bufs=4))     # [128,<=2048] weight cols
    rot = ctx.enter_context(tc.tile_pool(name="rot", bufs=3))       # rotating temps

    # ---- constants ----
    m0, m1 = _masks_np()
    mask_t = [nc.inline_tensor(m0, name="gmask0"), nc.inline_tensor(m1, name="gmask1")]
    masks = []
    for i in (0, 1):
        mt = cpool.tile([128, 256], F32, tag=f"mask{i}")
        nc.sync.dma_start(mt[:], mask_t[i][:, :])
        masks.append(mt)
    ones32f = cpool.tile([1, 32], F32, tag="ones32f")
    nc.vector.memset(ones32f[:], 1.0)
    ones128f = cpool.tile([1, 128], F32, tag="ones128f")
    nc.vector.memset(ones128f[:], 1.0)
    onesb = cpool.tile([128, 1], BF16, tag="onesb")
    nc.vector.memset(onesb[:], 1.0)

    # ---- stage 1: mlp_in ----
    # xT sbuf: [128, (kt8, b32)]
    xT = apool.tile([128, SEQ * BC], BF16, tag="xT")
    nc.sync.dma_start(xT[:], ins["xT"][:, :])
    b_in1 = cpool.tile([128, 64], F32, tag="b_in1")
    nc.sync.dma_start(b_in1[:], ins["b_in1"][:, :])
    b_in2 = cpool.tile([128, 64], F32, tag="b_in2")
    nc.sync.dma_start(b_in2[:], ins["b_in2"][:, :])

    # GEMM1: h1T[f,b] = W1[:,f].T @ xT ; 64 m-tiles, 8 k-tiles
    pbig_cm = tc.tile_pool(name="pbig", bufs=1, space="PSUM")
    pbig = pbig_cm.__enter__()
    ps1 = pbig.tile([128, 64 * BC], F32, tag="psbig")
    for mt in range(64):
        w1c = kcol.tile([128, 8 * 128], BF16, tag="kcol")
        nc.sync.dma_start(w1c[:], ins["w_in1"][mt])
        sl = ps1[:, mt * BC:(mt + 1) * BC]
        for kt in range(8):
            nc.tensor.matmul(sl, w1c[:, kt * 128:(kt + 1) * 128],
                             xT[:, kt * BC:(kt + 1) * BC],
                             start=(kt == 0), stop=(kt == 7))
    nc.vector.tensor_add(
        _r(ps1[:, :], "p (mt b) -> p mt b", mt=64),
        _r(ps1[:, :], "p (mt b) -> p mt b", mt=64),
        b_in1[:, :].unsqueeze(2).broadcast_to((128, 64, BC)))
    h1T = apool.tile([128, 64 * BC], BF16, tag="h1T")
    nc.scalar.activation(h1T[:], ps1[:], AF.Relu)

    # GEMM2: h2T = W2[:,f].T @ h1 ; psum free order (mt=(s,j), b) -> h2cm free (j,b,s)
    ps2 = pbig.tile([128, 64 * BC], F32, tag="psbig")
    for mt in range(64):
        w2c = wbig.tile([128, 64 * 128], BF16, tag="wbig")
        for ch in range(4):
            nc.sync.dma_start(w2c[:, ch * 2048:(ch + 1) * 2048],
                              ins["w_in2"][mt][:, ch * 2048:(ch + 1) * 2048])
        sl = ps2[:, mt * BC:(mt + 1) * BC]
        for kt in range(64):
            nc.tensor.matmul(sl, w2c[:, kt * 128:(kt + 1) * 128],
                             h1T[:, kt * BC:(kt + 1) * BC],
                             start=(kt == 0), stop=(kt == 63))
    nc.vector.tensor_add(
        _r(ps2[:, :], "p (mt b) -> p mt b", mt=64),
        _r(ps2[:, :], "p (mt b) -> p mt b", mt=64),
        b_in2[:, :].unsqueeze(2).broadcast_to((128, 64, BC)))
    # h2cm[p, (j, b, s)] <- ps2[p, (s, j, b)]
    h2cm = apool.tile([128, 8 * NT], BF16, tag="h2cm")
    nc.vector.tensor_copy(
        _r(h2cm[:, :], "p (j b s) -> p j b s", j=8, b=BC),
        _r(ps2[:, :], "p (s j b) -> p s j b", s=8, j=8).transpose([0, 2, 3, 1]))

    pbig_cm.__exit__(None, None, None)
    hin = h2cm
    for li in (1, 2):
        hin = mamba_layer(tc, ctx, ins, f"m{li}_", hin,
                          cpool, apool, kcol, rot,
                          masks, ones32f, ones128f, onesb)

    # ---- stage 3: mlp_out ----
    b_out1 = cpool.tile([128, 8], F32, tag="b_out1")
    nc.sync.dma_start(b_out1[:], ins["b_out1"][:, :])
    b_out2 = cpool.tile([REWARD, 1], F32, tag="b_out2")
    nc.sync.dma_start(b_out2[:], ins["b_out2"][:, :])
    h4v = _r(hin[:, :], "p (j b s) -> p j b s", j=8, b=BC)

    pout_cm = tc.tile_pool(name="pout", bufs=1, space="PSUM")
    pout = pout_cm.__enter__()
    ps3 = pout.tile([128, 8 * BC], F32, tag="ps3")
    for mt in range(8):
        woc = wbig.tile([128, 64 * 128], BF16, tag="wbig")
        for ch in range(4):
            nc.sync.dma_start(woc[:, ch * 2048:(ch + 1) * 2048],
                              ins["w_out1"][mt][:, ch * 2048:(ch + 1) * 2048])
        sl = ps3[:, mt * BC:(mt + 1) * BC]
        for kt in range(64):
            s, j = kt // 8, kt % 8
            nc.tensor.matmul(sl, woc[:, kt * 128:(kt + 1) * 128],
                             h4v[:, j, :, s], start=(kt == 0), stop=(kt == 63))
    nc.vector.tensor_add(
        _r(ps3[:, :], "p (mt b) -> p mt b", mt=8),
        _r(ps3[:, :], "p (mt b) -> p mt b", mt=8),
        b_out1[:, :].unsqueeze(2).broadcast_to((128, 8, BC)))
    h5 = apool.tile([128, 8 * BC], BF16, tag="h5")
    nc.scalar.activation(h5[:], ps3[:], AF.Relu)

    wo2 = cpool.tile([128, 8 * REWARD], BF16, tag="wo2")
    nc.sync.dma_start(wo2[:], ins["w_out2"][:, :])
    ps4 = pout.tile([REWARD, BC], F32, tag="ps4")
    for kt in range(8):
        nc.tensor.matmul(ps4[:], wo2[:, kt * REWARD:(kt + 1) * REWARD],
                         h5[:, kt * BC:(kt + 1) * BC], start=(kt == 0), stop=(kt == 7))
    outT = apool.tile([REWARD, BC], F32, tag="outT")
    nc.vector.tensor_scalar_add(outT[:], ps4[:], b_out2[:, 0:1])
    nc.sync.dma_start(out_ap[:, :], outT[:])
    pout_cm.__exit__(None, None, None)


def mamba_layer(tc, ctx, ins, pfx, hin, cpool, apool, kcol, rot,
                masks, ones32f, ones128f, onesb):
    nc = tc.nc
    li = pfx  # tag prefix
    pp_cm = tc.tile_pool(name=pfx + "psum", bufs=1, space="PSUM")
    pp = pp_cm.__enter__()

    convw = cpool.tile([128, 18 * DCONV], F32, tag=li + "convw")
    nc.sync.dma_start(convw[:], ins[pfx + "convw"][:, :])
    convb = cpool.tile([128, 18], F32, tag=li + "convb")
    nc.sync.dma_start(convb[:], ins[pfx + "convb"][:, :])
    dtb = cpool.tile([NH, 1], F32, tag=li + "dtb")
    nc.sync.dma_start(dtb[:], ins[pfx + "dtb"][:, :])
    aneg = cpool.tile([NH, 1], F32, tag=li + "aneg")
    nc.sync.dma_start(aneg[:], ins[pfx + "aneg"][:, :])
    dvec = cpool.tile([NH, 1], F32, tag=li + "dvec")
    nc.sync.dma_start(dvec[:], ins[pfx + "dvec"][:, :])
    normw = cpool.tile([128, 16], F32, tag=li + "normw")
    nc.sync.dma_start(normw[:], ins[pfx + "normw"][:, :])

    zbf = apool.tile([128, 16 * NT], BF16, tag="mb_zbf")
    xact = apool.tile([128, 18 * NT], BF16, tag="mb_xact")
    dt_raw = apool.tile([NH, NT], F32, tag="mb_dt_raw")

    # in_proj GEMM: m-tile order B,C,dt then x tiles then z tiles
    order = [32, 33, 34] + list(range(16, 32)) + list(range(16))
    for mt in order:
        mcols = 128 if mt != 34 else NH
        wc = kcol.tile([128, 8 * 128], BF16, tag="kcol")
        nc.sync.dma_start(wc[:], ins[pfx + "inproj"][mt])
        ps = pp.tile([128, NT], F32, tag="psin" + str(mt % 2))
        pss = ps[0:mcols, :]
        for kt in range(8):
            nc.tensor.matmul(pss, wc[:, kt * 128:kt * 128 + mcols],
                             hin[:, kt * NT:(kt + 1) * NT],
                             start=(kt == 0), stop=(kt == 7))
        if mt < 16:
            nc.scalar.copy(zbf[:, mt * NT:(mt + 1) * NT], ps[:])
        elif mt == 34:
            nc.scalar.copy(dt_raw[:], pss)
        else:
            i = mt - 16
            # conv + silu for this channel tile
            xbc = rot.tile([128, NT], F32, tag="xbc")
            nc.scalar.copy(xbc[:], ps[:])
            cw = convw[:, i * DCONV:(i + 1) * DCONV]
            cb = convb[:, i:i + 1]
            conv = rot.tile([128, NT], F32, tag="conv")
            xv = _r(xbc[:, :], "p (b s) -> p b s", b=BC)
            cv = _r(conv[:, :], "p (b s) -> p b s", b=BC)
            nc.vector.tensor_scalar(conv[:], xbc[:], cw[:, 3:4], cb,
                                    ALU.mult, ALU.add)
            for k in (2, 1, 0):
                d = 3 - k
                nc.vector.scalar_tensor_tensor(
                    cv[:, :, d:8], xv[:, :, 0:8 - d], cw[:, k:k + 1],
                    cv[:, :, d:8], ALU.mult, ALU.add)
            sg = rot.tile([128, NT], F32, tag="sg")
            nc.scalar.activation(sg[:], conv[:], AF.Sigmoid)
            nc.vector.tensor_mul(xact[:, i * NT:(i + 1) * NT], conv[:], sg[:])

    # ---- dt path ----
    dt = apool.tile([NH, NT], F32, tag="mb_dt")
    nc.scalar.activation(dt[:], dt_raw[:], AF.Exp, bias=dtb[:, 0:1])
    nc.scalar.activation(dt[:], dt[:], AF.Ln, bias=1.0)       # softplus
    lndt = apool.tile([NH, NT], F32, tag="mb_lndt")
    nc.scalar.activation(lndt[:], dt[:], AF.Ln)
    la = apool.tile([NH, NT], F32, tag="mb_la")
    nc.vector.tensor_scalar_mul(la[:], dt[:], aneg[:, 0:1])
    # cumsum along s (within b-groups of 8)
    lav = _r(la[:, :], "p (b s) -> p b s", b=BC)
    t1 = apool.tile([NH, NT], F32, tag="mb_cum1")
    t1v = _r(t1[:, :], "p (b s) -> p b s", b=BC)
    nc.vector.tensor_copy(t1v[:, :, 0:1], lav[:, :, 0:1])
    nc.vector.tensor_add(t1v[:, :, 1:8], lav[:, :, 1:8], lav[:, :, 0:7])
    t2 = apool.tile([NH, NT], F32, tag="mb_cum2")
    t2v = _r(t2[:, :], "p (b s) -> p b s", b=BC)
    nc.vector.tensor_copy(t2v[:, :, 0:2], t1v[:, :, 0:2])
    nc.vector.tensor_add(t2v[:, :, 2:8], t1v[:, :, 2:8], t1v[:, :, 0:6])
    cumla = apool.tile([NH, NT], F32, tag="mb_cumla")
    cv_ = _r(cumla[:, :], "p (b s) -> p b s", b=BC)
    nc.vector.tensor_copy(cv_[:, :, 0:4], t2v[:, :, 0:4])
    nc.vector.tensor_add(cv_[:, :, 4:8], t2v[:, :, 4:8], t2v[:, :, 0:4])
    cumlaA = apool.tile([NH, NT], F32, tag="mb_cumlaA")
    nc.vector.tensor_sub(cumlaA[:], cumla[:], lndt[:])

    # ---- G via PE gram + masked grouped reduce ----
    xB = xact[:, 16 * NT:17 * NT]
    xC = xact[:, 17 * NT:18 * NT]
    gh = []
    for half in (0, 1):
        psg = pp.tile([128, NT], F32, tag="psg")
        nc.tensor.matmul(psg[:], xC[:, half * 128:(half + 1) * 128], xB,
                         start=True, stop=True)
        mg = rot.tile([128, NT], F32, tag="mg")
        nc.vector.tensor_mul(mg[:], psg[:], masks[half][:])
        g = rot.tile([128, 8], F32, tag="ghalf")
        nc.vector.tensor_reduce(g[:], _r(mg[:, :], "p (b s) -> p s b", b=BC),
                                AX.X, ALU.add)
        gh.append(g)
    # flip G[(b,t),s] -> row [1,(b,t,s)] via SBUF->SBUF DMA flatten
    grow = apool.tile([1, 2048], F32, tag="mb_grow")
    for half in (0, 1):
        nc.sync.dma_start(grow[0:1, half * 1024:(half + 1) * 1024], gh[half][:, :])
    # replicate to 32 partitions via PE rank-1: psum[32,512] = ones32f.T @ grow-slice
    gb = apool.tile([NH, 2048], F32, tag="mb_gb")
    for q in range(4):
        pb = pp.tile([NH, 512], F32, tag="pbc")
        nc.tensor.matmul(pb[:], ones32f[0:1, :], grow[0:1, q * 512:(q + 1) * 512],
                         start=True, stop=True)
        nc.scalar.copy(gb[:, q * 512:(q + 1) * 512], pb[:])

    # ---- E = exp(cumla_t - cumlaA_s), W = E*G (+D diag), replicate -> Wexp ----
    w32 = apool.tile([NH, 2048], F32, tag="mb_w32")
    cb_t = _r(cumla[:, :], "p (b s) -> p b s", b=BC).unsqueeze(3).broadcast_to((NH, BC, 8, 8))
    ca_s = _r(cumlaA[:, :], "p (b s) -> p b s", b=BC).unsqueeze(2).broadcast_to((NH, BC, 8, 8))
    nc.vector.tensor_sub(_r(w32[:, :], "p (b t s) -> p b t s", b=BC, t=8), cb_t, ca_s)
    nc.vector.tensor_scalar_min(w32[:], w32[:], 30.0)  # padded t<s pairs would overflow exp
    nc.scalar.activation(w32[:], w32[:], AF.Exp)
    nc.vector.tensor_mul(w32[:], w32[:], gb[:])
    w32v = _r(w32[:, :], "p (b t s) -> p b t s", b=BC, t=8)
    for t in range(8):
        nc.vector.tensor_scalar_add(w32v[:, :, t:t + 1, t:t + 1],
                                    w32v[:, :, t:t + 1, t:t + 1], dvec[:, 0:1])
    wexp = apool.tile([128, 2048], BF16, tag="mb_wexp")
    nc.vector.tensor_copy(wexp[0:32, :], w32[:])
    nc.vector.tensor_copy(wexp[32:64, :], wexp[0:32, :])
    nc.vector.tensor_copy(wexp[64:128, :], wexp[0:64, :])

    # ---- scan: per x channel tile: Y = reduce_s( Xbcast * Wexp ) ----
    y2 = apool.tile([128, 16 * NT], F32, tag="mb_y2")
    psum_ss = pp.tile([1, NT], F32, tag="pss")
    for i in range(16):
        xi = xact[:, i * NT:(i + 1) * NT]
        xbc_ap = _r(xi, "p (b s) -> p b s", b=BC).unsqueeze(2).broadcast_to((128, BC, 8, 8))
        tmp = rot.tile([128, 2048], BF16, tag="scantmp")
        nc.vector.tensor_mul(_r(tmp[:, :], "p (b t s) -> p b t s", b=BC, t=8),
                             xbc_ap, _r(wexp[:, :], "p (b t s) -> p b t s", b=BC, t=8))
        yi = rot.tile([128, NT], F32, tag="yi")
        nc.vector.tensor_reduce(yi[:], _r(tmp[:, :], "p (b t s) -> p b t s", b=BC, t=8),
                                AX.X, ALU.add)
        # gating: y * z * sigmoid(z)
        zi = zbf[:, i * NT:(i + 1) * NT]
        sgz = rot.tile([128, NT], BF16, tag="sgz")
        nc.scalar.activation(sgz[:], zi, AF.Sigmoid)
        y2i = y2[:, i * NT:(i + 1) * NT]
        nc.vector.tensor_mul(yi[:], yi[:], zi)
        nc.vector.tensor_mul(y2i, yi[:], sgz[:])
        ybf = rot.tile([128, NT], BF16, tag="ybf")
        nc.vector.tensor_mul(ybf[:], y2i, y2i)
        nc.tensor.matmul(psum_ss[:], onesb[:, :], ybf[:],
                         start=(i == 0), stop=(i == 15))
    # Note: ybf = y2^2 cast to bf16; summed over channels via ones-matmul.
    epsb = cpool.tile([1, 1], F32, tag=li + "epsb")
    nc.vector.memset(epsb[:], 1e-5)
    rsq = apool.tile([1, NT], F32, tag="mb_rsq")
    nc.scalar.activation(rsq[:], psum_ss[:], AF.Sqrt, bias=epsb[0:1, 0:1], scale=1.0 / DIN)
    nc.vector.reciprocal(rsq[:], rsq[:])
    rsqb = apool.tile([128, NT], F32, tag="mb_rsqb")
    prs = pp.tile([128, NT], F32, tag="psg")
    nc.tensor.matmul(prs[:], ones128f[0:1, :], rsq[0:1, :], start=True, stop=True)
    nc.scalar.copy(rsqb[:], prs[:])
    ynorm = apool.tile([128, 16 * NT], BF16, tag="mb_ynorm")
    for i in range(16):
        nc.vector.scalar_tensor_tensor(
            ynorm[:, i * NT:(i + 1) * NT], y2[:, i * NT:(i + 1) * NT],
            normw[:, i:i + 1], rsqb[:], ALU.mult, ALU.mult)

    # ---- out_proj ----
    hout = apool.tile([128, 8 * NT], BF16, tag=li + "hout")
    for mt in range(8):
        wc = kcol.tile([128, 16 * 128], BF16, tag="kcolw")
        nc.sync.dma_start(wc[:], ins[pfx + "outproj"][mt])
        ps = pp.tile([128, NT], F32, tag="psout" + str(mt % 2))
        for kt in range(16):
            nc.tensor.matmul(ps[:], wc[:, kt * 128:(kt + 1) * 128],
                             ynorm[:, kt * NT:(kt + 1) * NT],
                             start=(kt == 0), stop=(kt == 15))
        nc.scalar.copy(hout[:, mt * NT:(mt + 1) * NT], ps[:])
    pp_cm.__exit__(None, None, None)
    return hout


_CACHE = {}


def _build():
    if "nc" in _CACHE:
        return _CACHE["nc"], _CACHE["innames"], _CACHE["outname"]
    in_specs = {
        "xT": ([128, SEQ * BC], BF16),
        "w_in1": ([64, 128, 1024], BF16), "b_in1": ([128, 64], F32),
        "w_in2": ([64, 128, 8192], BF16), "b_in2": ([128, 64], F32),
        "w_out1": ([8, 128, 8192], BF16), "b_out1": ([128, 8], F32),
        "w_out2": ([128, 8 * REWARD], BF16), "b_out2": ([REWARD, 1], F32),
    }
    for li in (1, 2):
        p = f"m{li}_"
        in_specs.update({
            p + "inproj": ([35, 128, 1024], BF16),
            p + "convw": ([128, 18 * DCONV], F32),
            p + "convb": ([128, 18], F32),
            p + "dtb": ([NH, 1], F32),
            p + "aneg": ([NH, 1], F32),
            p + "dvec": ([NH, 1], F32),
            p + "normw": ([128, 16], F32),
            p + "outproj": ([8, 128, 2048], BF16),
        })
    nc = bacc.Bacc("TRN2", target_bir_lowering=False, debug=False,
                   enable_asserts=True, num_devices=NCORES)
    ins = {}
    for name, (shape, dt) in in_specs.items():
        ins[name] = nc.dram_tensor(name, shape, dt, kind="ExternalInput").ap()
    out_ap = nc.dram_tensor("out", [REWARD, BC], F32, kind="ExternalOutput").ap()
    with tile.TileContext(nc) as tc:
        emit(tc, ins, out_ap)
    nc.compile()
    _CACHE["nc"] = nc
    _CACHE["innames"] = list(in_specs.keys())
    _CACHE["outname"] = "out"
    return nc, _CACHE["innames"], "out"


def kernel(**inputs) -> np.ndarray:
    nc, innames, outname = _build()
    in_maps = _host_prep(inputs)
    res = run_bass_kernel_spmd(nc, in_maps, core_ids=list(range(NCORES)))
    out = np.zeros((BATCH, REWARD), np.float32)
    for c in range(NCORES):
        out[c * BC:(c + 1) * BC, :] = np.asarray(res.results[c][outname]).T
    return out


if __name__ == "__main__":
    rng = np.random.default_rng(0)
    fake = {"x": rng.standard_normal((BATCH, HSS), dtype=np.float32)}
    print("smoke build only")
    _build()
    print("build ok")


# revision 19
# speedup vs baseline: 1.6090x; 1.0912x over previous
"""NeuralControlCritic Trainium2 kernel — 8-core SPMD, batch-data-parallel.

Network: h = relu(x@W1+b1)@W2+b2 -> reshape (B,8,1024) -> mamba2 x2 -> relu(h@Wo1+bo1)@Wo2+bo2
All activations on-chip are kept feature/channel-major: [features->partitions, (b,s)->free],
token column order col = b*SEQ + s.

Mamba scan uses the SSD O(L^2) form:
  Y[b,t,h,p] = sum_s W[b,h,t,s] * X[b,s,h,p],
  W = exp(cumla_t - (cumla_s - ln dt_s)) * G[b,t,s]  (+ D[h] on t==s diag)
  G[b,t,s] = sum_d C[b,t,d]*B[b,s,d]   (PE gram + diag-mask + grouped reduce)
The d_inner channel axis is host-permuted from (h,p) to (p,h) order so the scan's
per-(h,b) weight expands to channel tiles by pure partition-block replication.
"""
import numpy as np
import ml_dtypes
from contextlib import ExitStack

import concourse.bass as bass
import concourse.bacc as bacc
import concourse.mybir as mybir
import concourse.tile as tile
from concourse import library_config
from concourse.bass_utils import run_bass_kernel_spmd
from concourse._compat import with_exitstack

F32 = mybir.dt.float32
BF16 = mybir.dt.bfloat16
AF = mybir.ActivationFunctionType
ALU = mybir.AluOpType
AX = mybir.AxisListType
BF16NP = ml_dtypes.bfloat16

HSS = 1024
SEQ = 8
HID = 1024
REWARD = 2
BATCH = 256
DIN = 2048
HD = 64
NH = 32
DS = 128
DCONV = 4
CONVD = DIN + 2 * DS            # 2304
DPROJ = 2 * DIN + 2 * DS + NH   # 4384
NCORES = 8
BC = BATCH // NCORES            # 32 batch rows per core
NT = BC * SEQ                   # 256 tokens per core; col = b*SEQ + s

# channel permutation (h,p) -> (p,h): NEWPERM[p*NH+h] = h*HD+p
_PERM = np.arange(DIN).reshape(NH, HD).T.reshape(-1)  # new c = p*32+h maps to old h*64+p


def _perm_cols_z_x(in_proj):
    """Permute z and x column blocks of in_proj to (p,h) channel order."""
    w = np.array(in_proj, dtype=np.float32, copy=True)
    w[:, 0:DIN] = w[:, 0:DIN][:, _PERM]
    w[:, DIN:2 * DIN] = w[:, DIN:2 * DIN][:, _PERM]
    return w


def _host_prep(inputs):
    """Build the per-core input maps (all host-side transforms are free)."""
    f32 = lambda a: np.ascontiguousarray(np.asarray(a, dtype=np.float32))
    bf = lambda a: np.ascontiguousarray(np.asarray(a, dtype=np.float32).astype(BF16NP))

    def wtiles(w, kt, mt):
        # (K, M) -> [mt, 128(p), kt*128(j-major? no: kt blocks of j)] with
        # out[m, p, k*128:(k+1)*128] = w[k*128+p, m*128:(m+1)*128]
        K, M = w.shape
        a = np.asarray(w, dtype=np.float32).reshape(kt, 128, mt, M // mt)
        a = a.transpose(2, 1, 0, 3).reshape(mt, 128, kt * (M // mt))
        return np.ascontiguousarray(a.astype(BF16NP))

    common = {}
    common["w_in1"] = wtiles(inputs["w_in1"], 8, 64)
    common["b_in1"] = np.ascontiguousarray(f32(inputs["b_in1"]).reshape(64, 128).T)
    common["w_in2"] = wtiles(inputs["w_in2"], 64, 64)
    common["b_in2"] = np.ascontiguousarray(f32(inputs["b_in2"]).reshape(64, 128).T)
    for li in (1, 2):
        p = f"m{li}_"
        ipp = _perm_cols_z_x(inputs[p + "in_proj"])
        ipp = np.concatenate([ipp, np.zeros((HSS, 4480 - DPROJ), np.float32)], axis=1)
        common[p + "inproj"] = wtiles(ipp, 8, 35)
        cw = f32(inputs[p + "conv_w"])
        cb = f32(inputs[p + "conv_b"])
        cw = np.concatenate([cw[0:DIN][_PERM], cw[DIN:]], axis=0)
        cb = np.concatenate([cb[0:DIN][_PERM], cb[DIN:]], axis=0)
        common[p + "convw"] = np.ascontiguousarray(
            cw.reshape(18, 128, DCONV).transpose(1, 0, 2).reshape(128, 18 * DCONV))
        common[p + "convb"] = np.ascontiguousarray(
            cb.reshape(18, 128).T)                                # [128, 18] f32
        common[p + "dtb"] = f32(inputs[p + "dt_bias"]).reshape(NH, 1)
        common[p + "aneg"] = (-np.exp(f32(inputs[p + "A_log"]))).reshape(NH, 1)
        common[p + "dvec"] = f32(inputs[p + "D"]).reshape(NH, 1)
        common[p + "normw"] = np.ascontiguousarray(
            f32(inputs[p + "norm_w"])[_PERM].reshape(16, 128).T)  # [128, 16] f32
        common[p + "outproj"] = wtiles(np.array(inputs[p + "out_proj"])[_PERM, :], 16, 8)
    common["w_out1"] = wtiles(inputs["w_out1"], 64, 8)
    common["b_out1"] = np.ascontiguousarray(f32(inputs["b_out1"]).reshape(8, 128).T)
    common["w_out2"] = np.ascontiguousarray(
        f32(inputs["w_out2"]).reshape(8, 128, REWARD).transpose(1, 0, 2)
        .reshape(128, 8 * REWARD).astype(BF16NP))
    common["b_out2"] = f32(inputs["b_out2"]).reshape(REWARD, 1)

    x = np.asarray(inputs["x"], dtype=np.float32)
    in_maps = []
    for c in range(NCORES):
        m = dict(common)
        xt = x[c * BC:(c + 1) * BC, :].T.reshape(8, 128, BC).transpose(1, 0, 2)
        m["xT"] = np.ascontiguousarray(xt.reshape(128, SEQ * BC).astype(BF16NP))
        in_maps.append(m)
    return in_maps


def _masks_np():
    """G-gram diag masks with causality baked in: mask_h[(b16,t8),(b',s)]=(b'==b+16h)&(t>=s)."""
    masks = []
    for half in (0, 1):
        m = np.zeros((128, 256), np.float32)
        for r in range(128):
            b, t = r // 8 + 16 * half, r % 8
            for c in range(256):
                bp, s = c // 8, c % 8
                if bp == b and t >= s:
                    m[r, c] = 1.0
        masks.append(m)
    return masks


def _r(ap, spec, **kw):
    return ap.rearrange(spec, **kw)


@with_exitstack
def emit(ctx: ExitStack, tc: "tile.TileContext", ins: dict, out_ap):
    nc = tc.nc

    cpool = ctx.enter_context(tc.tile_pool(name="consts", bufs=1))
    apool = ctx.enter_context(tc.tile_pool(name="acts", bufs=1))
    wbig = ctx.enter_context(tc.tile_pool(name="wbig", bufs=3))     # [128,8192] streams
    kcol = ctx.enter_context(tc.tile_pool(name="kcol", bufs=4))     # [128,<=2048] weight cols
    rot = ctx.enter_context(tc.tile_pool(name="rot", bufs=3))  # rotating temps

    # ---- constants ----
    m0, m1 = _masks_np()
    mask_t = [nc.inline_tensor(m0, name="gmask0"), nc.inline_tensor(m1, name="gmask1")]
    masks = []
    for i in (0, 1):
        mt = cpool.tile([128, 256], F32, tag=f"mask{i}")
        nc.sync.dma_start(mt[:], mask_t[i][:, :])
        masks.append(mt)
    ones32f = cpool.tile([1, 32], F32, tag="ones32f")
    nc.vector.memset(ones32f[:], 1.0)
    ones128f = cpool.tile([1, 128], F32, tag="ones128f")
    nc.vector.memset(ones128f[:], 1.0)
    onesb = cpool.tile([128, 1], BF16, tag="onesb")
    nc.vector.memset(onesb[:], 1.0)

    # ---- stage 1: mlp_in ----
    # xT sbuf: [128, (kt8, b32)]
    xT = apool.tile([128, SEQ * BC], BF16, tag="xT")
    nc.sync.dma_start(xT[:], ins["xT"][:, :])
    b_in1 = cpool.tile([128, 64], F32, tag="b_in1")
    nc.sync.dma_start(b_in1[:], ins["b_in1"][:, :])
    b_in2 = cpool.tile([128, 64], F32, tag="b_in2")
    nc.sync.dma_start(b_in2[:], ins["b_in2"][:, :])

    # GEMM1: h1T[f,b] = W1[:,f].T @ xT ; 64 m-tiles, 8 k-tiles
    pbig_cm = tc.tile_pool(name="pbig", bufs=1, space="PSUM")
    pbig = pbig_cm.__enter__()
    ps1 = pbig.tile([128, 64 * BC], F32, tag="psbig")
    for mt in range(64):
        w1c = kcol.tile([128, 8 * 128], BF16, tag="kcol")
        nc.sync.dma_start(w1c[:], ins["w_in1"][mt])
        sl = ps1[:, mt * BC:(mt + 1) * BC]
        for kt in range(8):
            nc.tensor.matmul(sl, w1c[:, kt * 128:(kt + 1) * 128],
                             xT[:, kt * BC:(kt + 1) * BC],
                             start=(kt == 0), stop=(kt == 7))
    nc.vector.tensor_add(
        _r(ps1[:, :], "p (mt b) -> p mt b", mt=64),
        _r(ps1[:, :], "p (mt b) -> p mt b", mt=64),
        b_in1[:, :].unsqueeze(2).broadcast_to((128, 64, BC)))
    h1T = apool.tile([128, 64 * BC], BF16, tag="h1T")
    nc.scalar.activation(h1T[:], ps1[:], AF.Relu)

    # GEMM2: h2T = W2[:,f].T @ h1 ; psum free order (mt=(s,j), b) -> h2cm free (j,b,s)
    ps2 = pbig.tile([128, 64 * BC], F32, tag="psbig")
    for mt in range(64):
        w2c = wbig.tile([128, 64 * 128], BF16, tag="wbig")
        for ch in range(4):
            nc.sync.dma_start(w2c[:, ch * 2048:(ch + 1) * 2048],
                              ins["w_in2"][mt][:, ch * 2048:(ch + 1) * 2048])
        sl = ps2[:, mt * BC:(mt + 1) * BC]
        for kt in range(64):
            nc.tensor.matmul(sl, w2c[:, kt * 128:(kt + 1) * 128],
                             h1T[:, kt * BC:(kt + 1) * BC],
                             start=(kt == 0), stop=(kt == 63))
    nc.vector.tensor_add(
        _r(ps2[:, :], "p (mt b) -> p mt b", mt=64),
        _r(ps2[:, :], "p (mt b) -> p mt b", mt=64),
        b_in2[:, :].unsqueeze(2).broadcast_to((128, 64, BC)))
    # h2cm[p, (j, b, s)] <- ps2[p, (s, j, b)]
    h2cm = apool.tile([128, 8 * NT], BF16, tag="h2cm")
    nc.vector.tensor_copy(
        _r(h2cm[:, :], "p (j b s) -> p j b s", j=8, b=BC),
        _r(ps2[:, :], "p (s j b) -> p s j b", s=8, j=8).transpose([0, 2, 3, 1]))

    pbig_cm.__exit__(None, None, None)
    hin = h2cm
    for li in (1, 2):
        hin = mamba_layer(tc, ctx, ins, f"m{li}_", hin,
                          cpool, apool, kcol, rot,
                          masks, ones32f, ones128f, onesb)

    # ---- stage 3: mlp_out ----
    b_out1 = cpool.tile([128, 8], F32, tag="b_out1")
    nc.sync.dma_start(b_out1[:], ins["b_out1"][:, :])
    b_out2 = cpool.tile([REWARD, 1], F32, tag="b_out2")
    nc.sync.dma_start(b_out2[:], ins["b_out2"][:, :])
    h4v = _r(hin[:, :], "p (j b s) -> p j b s", j=8, b=BC)

    pout_cm = tc.tile_pool(name="pout", bufs=1, space="PSUM")
    pout = pout_cm.__enter__()
    ps3 = pout.tile([128, 8 * BC], F32, tag="ps3")
    for mt in range(8):
        woc = wbig.tile([128, 64 * 128], BF16, tag="wbig")
        for ch in range(4):
            nc.sync.dma_start(woc[:, ch * 2048:(ch + 1) * 2048],
                              ins["w_out1"][mt][:, ch * 2048:(ch + 1) * 2048])
        sl = ps3[:, mt * BC:(mt + 1) * BC]
        for kt in range(64):
            s, j = kt // 8, kt % 8
            nc.tensor.matmul(sl, woc[:, kt * 128:(kt + 1) * 128],
                             h4v[:, j, :, s], start=(kt == 0), stop=(kt == 63))
    nc.vector.tensor_add(
        _r(ps3[:, :], "p (mt b) -> p mt b", mt=8),
        _r(ps3[:, :], "p (mt b) -> p mt b", mt=8),
        b_out1[:, :].unsqueeze(2).broadcast_to((128, 8, BC)))
    h5 = apool.tile([128, 8 * BC], BF16, tag="h5")
    nc.scalar.activation(h5[:], ps3[:], AF.Relu)

    wo2 = cpool.tile([128, 8 * REWARD], BF16, tag="wo2")
    nc.sync.dma_start(wo2[:], ins["w_out2"][:, :])
    ps4 = pout.tile([REWARD, BC], F32, tag="ps4")
    for kt in range(8):
        nc.tensor.matmul(ps4[:], wo2[:, kt * REWARD:(kt + 1) * REWARD],
                         h5[:, kt * BC:(kt + 1) * BC], start=(kt == 0), stop=(kt == 7))
    outT = apool.tile([REWARD, BC], F32, tag="outT")
    nc.vector.tensor_scalar_add(outT[:], ps4[:], b_out2[:, 0:1])
    nc.sync.dma_start(out_ap[:, :], outT[:])
    pout_cm.__exit__(None, None, None)


def mamba_layer(tc, ctx, ins, pfx, hin, cpool, apool, kcol, rot,
                masks, ones32f, ones128f, onesb):
    nc = tc.nc
    li = pfx  # tag prefix
    pp_cm = tc.tile_pool(name=pfx + "psum", bufs=1, space="PSUM")
    pp = pp_cm.__enter__()

    convw = cpool.tile([128, 18 * DCONV], F32, tag=li + "convw")
    nc.sync.dma_start(convw[:], ins[pfx + "convw"][:, :])
    convb = cpool.tile([128, 18], F32, tag=li + "convb")
    nc.sync.dma_start(convb[:], ins[pfx + "convb"][:, :])
    dtb = cpool.tile([NH, 1], F32, tag=li + "dtb")
    nc.sync.dma_start(dtb[:], ins[pfx + "dtb"][:, :])
    aneg = cpool.tile([NH, 1], F32, tag=li + "aneg")
    nc.sync.dma_start(aneg[:], ins[pfx + "aneg"][:, :])
    dvec = cpool.tile([NH, 1], F32, tag=li + "dvec")
    nc.sync.dma_start(dvec[:], ins[pfx + "dvec"][:, :])
    normw = cpool.tile([128, 16], F32, tag=li + "normw")
    nc.sync.dma_start(normw[:], ins[pfx + "normw"][:, :])

    zbf = apool.tile([128, 16 * NT], BF16, tag="mb_zbf")
    xact = apool.tile([128, 18 * NT], BF16, tag="mb_xact")
    dt_raw = apool.tile([NH, NT], F32, tag="mb_dt_raw")

    # in_proj GEMM: m-tile order B,C,dt then x tiles then z tiles
    order = [32, 33, 34] + list(range(16, 32)) + list(range(16))
    for mt in order:
        mcols = 128 if mt != 34 else NH
        wc = kcol.tile([128, 8 * 128], BF16, tag="kcol")
        nc.sync.dma_start(wc[:], ins[pfx + "inproj"][mt])
        ps = pp.tile([128, NT], F32, tag="psin" + str(mt % 2))
        pss = ps[0:mcols, :]
        for kt in range(8):
            nc.tensor.matmul(pss, wc[:, kt * 128:kt * 128 + mcols],
                             hin[:, kt * NT:(kt + 1) * NT],
                             start=(kt == 0), stop=(kt == 7))
        if mt < 16:
            nc.scalar.copy(zbf[:, mt * NT:(mt + 1) * NT], ps[:])
        elif mt == 34:
            nc.scalar.copy(dt_raw[:], pss)
        else:
            i = mt - 16
            # conv + silu for this channel tile
            xbc = rot.tile([128, NT], F32, tag="xbc")
            nc.scalar.copy(xbc[:], ps[:])
            cw = convw[:, i * DCONV:(i + 1) * DCONV]
            cb = convb[:, i:i + 1]
            conv = rot.tile([128, NT], F32, tag="conv")
            xv = _r(xbc[:, :], "p (b s) -> p b s", b=BC)
            cv = _r(conv[:, :], "p (b s) -> p b s", b=BC)
            nc.vector.tensor_scalar(conv[:], xbc[:], cw[:, 3:4], cb,
                                    ALU.mult, ALU.add)
            for k in (2, 1, 0):
                d = 3 - k
                nc.vector.scalar_tensor_tensor(
                    cv[:, :, d:8], xv[:, :, 0:8 - d], cw[:, k:k + 1],
                    cv[:, :, d:8], ALU.mult, ALU.add)
            sg = rot.tile([128, NT], F32, tag="sg")
            nc.scalar.activation(sg[:], conv[:], AF.Sigmoid)
            nc.vector.tensor_mul(xact[:, i * NT:(i + 1) * NT], conv[:], sg[:])

    # ---- dt path ----
    dt = apool.tile([NH, NT], F32, tag="mb_dt")
    nc.scalar.activation(dt[:], dt_raw[:], AF.Exp, bias=dtb[:, 0:1])
    nc.scalar.activation(dt[:], dt[:], AF.Ln, bias=1.0)       # softplus
    lndt = apool.tile([NH, NT], F32, tag="mb_lndt")
    nc.scalar.activation(lndt[:], dt[:], AF.Ln)
    la = apool.tile([NH, NT], F32, tag="mb_la")
    nc.vector.tensor_scalar_mul(la[:], dt[:], aneg[:, 0:1])
    # cumsum along s (within b-groups of 8)
    lav = _r(la[:, :], "p (b s) -> p b s", b=BC)
    t1 = apool.tile([NH, NT], F32, tag="mb_cum1")
    t1v = _r(t1[:, :], "p (b s) -> p b s", b=BC)
    nc.vector.tensor_copy(t1v[:, :, 0:1], lav[:, :, 0:1])
    nc.vector.tensor_add(t1v[:, :, 1:8], lav[:, :, 1:8], lav[:, :, 0:7])
    t2 = apool.tile([NH, NT], F32, tag="mb_cum2")
    t2v = _r(t2[:, :], "p (b s) -> p b s", b=BC)
    nc.vector.tensor_copy(t2v[:, :, 0:2], t1v[:, :, 0:2])
    nc.vector.tensor_add(t2v[:, :, 2:8], t1v[:, :, 2:8], t1v[:, :, 0:6])
    cumla = apool.tile([NH, NT], F32, tag="mb_cumla")
    cv_ = _r(cumla[:, :], "p (b s) -> p b s", b=BC)
    nc.vector.tensor_copy(cv_[:, :, 0:4], t2v[:, :, 0:4])
    nc.vector.tensor_add(cv_[:, :, 4:8], t2v[:, :, 4:8], t2v[:, :, 0:4])
    cumlaA = apool.tile([NH, NT], F32, tag="mb_cumlaA")
    nc.vector.tensor_sub(cumlaA[:], cumla[:], lndt[:])

    # ---- G via PE gram + masked grouped reduce ----
    xB = xact[:, 16 * NT:17 * NT]
    xC = xact[:, 17 * NT:18 * NT]
    gh = []
    for half in (0, 1):
        psg = pp.tile([128, NT], F32, tag="psg")
        nc.tensor.matmul(psg[:], xC[:, half * 128:(half + 1) * 128], xB,
                         start=True, stop=True)
        mg = rot.tile([128, NT], F32, tag="mg")
        nc.vector.tensor_mul(mg[:], psg[:], masks[half][:])
        g = rot.tile([128, 8], F32, tag="ghalf")
        nc.vector.tensor_reduce(g[:], _r(mg[:, :], "p (b s) -> p s b", b=BC),
                                AX.X, ALU.add)
        gh.append(g)
    # flip G[(b,t),s] -> row [1,(b,t,s)] via SBUF->SBUF DMA flatten
    grow = apool.tile([1, 2048], F32, tag="mb_grow")
    for half in (0, 1):
        nc.sync.dma_start(grow[0:1, half * 1024:(half + 1) * 1024], gh[half][:, :])
    # replicate to 32 partitions via PE rank-1: psum[32,512] = ones32f.T @ grow-slice
    gb = apool.tile([NH, 2048], F32, tag="mb_gb")
    for q in range(4):
        pb = pp.tile([NH, 512], F32, tag="pbc")
        nc.tensor.matmul(pb[:], ones32f[0:1, :], grow[0:1, q * 512:(q + 1) * 512],
                         start=True, stop=True)
        nc.scalar.copy(gb[:, q * 512:(q + 1) * 512], pb[:])

    # ---- E = exp(cumla_t - cumlaA_s), W = E*G (+D diag), replicate -> Wexp ----
    w32 = apool.tile([NH, 2048], F32, tag="mb_w32")
    cb_t = _r(cumla[:, :], "p (b s) -> p b s", b=BC).unsqueeze(3).broadcast_to((NH, BC, 8, 8))
    ca_s = _r(cumlaA[:, :], "p (b s) -> p b s", b=BC).unsqueeze(2).broadcast_to((NH, BC, 8, 8))
    nc.vector.tensor_sub(_r(w32[:, :], "p (b t s) -> p b t s", b=BC, t=8), cb_t, ca_s)
    nc.vector.tensor_scalar_min(w32[:], w32[:], 30.0)  # padded t<s pairs would overflow exp
    nc.scalar.activation(w32[:], w32[:], AF.Exp)
    nc.vector.tensor_mul(w32[:], w32[:], gb[:])
    w32v = _r(w32[:, :], "p (b t s) -> p b t s", b=BC, t=8)
    for t in range(8):
        nc.vector.tensor_scalar_add(w32v[:, :, t:t + 1, t:t + 1],
                                    w32v[:, :, t:t + 1, t:t + 1], dvec[:, 0:1])
    wexp = apool.tile([128, 2048], BF16, tag="mb_wexp")
    nc.vector.tensor_copy(wexp[0:32, :], w32[:])
    nc.vector.tensor_copy(wexp[32:64, :], wexp[0:32, :])
    nc.vector.tensor_copy(wexp[64:128, :], wexp[0:64, :])

    # ---- scan: per x channel tile: Y = reduce_s( Xbcast * Wexp ) ----
    y2 = apool.tile([128, 16 * NT], F32, tag="mb_y2")
    psum_ss = pp.tile([1, NT], F32, tag="pss")
    for i in range(16):
        xi = xact[:, i * NT:(i + 1) * NT]
        xbc_ap = _r(xi, "p (b s) -> p b s", b=BC).unsqueeze(2).broadcast_to((128, BC, 8, 8))
        tmp = rot.tile([128, 2048], BF16, tag="scantmp")
        nc.vector.tensor_mul(_r(tmp[:, :], "p (b t s) -> p b t s", b=BC, t=8),
                             xbc_ap, _r(wexp[:, :], "p (b t s) -> p b t s", b=BC, t=8))
        yi = rot.tile([128, NT], F32, tag="yi")
        nc.vector.tensor_reduce(yi[:], _r(tmp[:, :], "p (b t s) -> p b t s", b=BC, t=8),
                                AX.X, ALU.add)
        # gating: y * z * sigmoid(z)
        zi = zbf[:, i * NT:(i + 1) * NT]
        sgz = rot.tile([128, NT], BF16, tag="sgz")
        nc.scalar.activation(sgz[:], zi, AF.Sigmoid)
        y2i = y2[:, i * NT:(i + 1) * NT]
        nc.vector.tensor_mul(yi[:], yi[:], zi)
        nc.vector.tensor_mul(y2i, yi[:], sgz[:])
        ybf = rot.tile([128, NT], BF16, tag="ybf")
        nc.vector.tensor_mul(ybf[:], y2i, y2i)
        nc.tensor.matmul(psum_ss[:], onesb[:, :], ybf[:],
                         start=(i == 0), stop=(i == 15))
    # Note: ybf = y2^2 cast to bf16; summed over channels via ones-matmul.
    epsb = cpool.tile([1, 1], F32, tag=li + "epsb")
    nc.vector.memset(epsb[:], 1e-5)
    rsq = apool.tile([1, NT], F32, tag="mb_rsq")
    nc.scalar.activation(rsq[:], psum_ss[:], AF.Sqrt, bias=epsb[0:1, 0:1], scale=1.0 / DIN)
    nc.vector.reciprocal(rsq[:], rsq[:])
    rsqb = apool.tile([128, NT], F32, tag="mb_rsqb")
    prs = pp.tile([128, NT], F32, tag="psg")
    nc.tensor.matmul(prs[:], ones128f[0:1, :], rsq[0:1, :], start=True, stop=True)
    nc.scalar.copy(rsqb[:], prs[:])
    ynorm = apool.tile([128, 16 * NT], BF16, tag="mb_ynorm")
    for i in range(16):
        nc.vector.scalar_tensor_tensor(
            ynorm[:, i * NT:(i + 1) * NT], y2[:, i * NT:(i + 1) * NT],
            normw[:, i:i + 1], rsqb[:], ALU.mult, ALU.mult)

    # ---- out_proj ----
    hout = apool.tile([128, 8 * NT], BF16, tag=li + "hout")
    for mt in range(8):
        wc = kcol.tile([128, 16 * 128], BF16, tag="kcolw")
        nc.sync.dma_start(wc[:], ins[pfx + "outproj"][mt])
        ps = pp.tile([128, NT], F32, tag="psout" + str(mt % 2))
        for kt in range(16):
            nc.tensor.matmul(ps[:], wc[:, kt * 128:(kt + 1) * 128],
                             ynorm[:, kt * NT:(kt + 1) * NT],
                             start=(kt == 0), stop=(kt == 15))
        nc.scalar.copy(hout[:, mt * NT:(mt + 1) * NT], ps[:])
    pp_cm.__exit__(None, None, None)
    return hout


_CACHE = {}


def _build():
    if "nc" in _CACHE:
        return _CACHE["nc"], _CACHE["innames"], _CACHE["outname"]
    in_specs = {
        "xT": ([128, SEQ * BC], BF16),
        "w_in1": ([64, 128, 1024], BF16), "b_in1": ([128, 64], F32),
        "w_in2": ([64, 128, 8192], BF16), "b_in2": ([128, 64], F32),
        "w_out1": ([8, 128, 8192], BF16), "b_out1": ([128, 8], F32),
        "w_out2": ([128, 8 * REWARD], BF16), "b_out2": ([REWARD, 1], F32),
    }
    for li in (1, 2):
        p = f"m{li}_"
        in_specs.update({
            p + "inproj": ([35, 128, 1024], BF16),
            p + "convw": ([128, 18 * DCONV], F32),
            p + "convb": ([128, 18], F32),
            p + "dtb": ([NH, 1], F32),
            p + "aneg": ([NH, 1], F32),
            p + "dvec": ([NH, 1], F32),
            p + "normw": ([128, 16], F32),
            p + "outproj": ([8, 128, 2048], BF16),
        })
    nc = bacc.Bacc("TRN2", target_bir_lowering=False, debug=False,
                   enable_asserts=True, num_devices=NCORES)
    ins = {}
    for name, (shape, dt) in in_specs.items():
        ins[name] = nc.dram_tensor(name, shape, dt, kind="ExternalInput").ap()
    out_ap = nc.dram_tensor("out", [REWARD, BC], F32, kind="ExternalOutput").ap()
    with tile.TileContext(nc) as tc:
        emit(tc, ins, out_ap)
    nc.compile()
    _CACHE["nc"] = nc
    _CACHE["innames"] = list(in_specs.keys())
    _CACHE["outname"] = "out"
    return nc, _CACHE["innames"], "out"


def kernel(**inputs) -> np.ndarray:
    nc, innames, outname = _build()
    in_maps = _host_prep(inputs)
    res = run_bass_kernel_spmd(nc, in_maps, core_ids=list(range(NCORES)))
    out = np.zeros((BATCH, REWARD), np.float32)
    for c in range(NCORES):
        out[c * BC:(c + 1) * BC, :] = np.asarray(res.results[c][outname]).T
    return out


if __name__ == "__main__":
    rng = np.random.default_rng(0)
    fake = {"x": rng.standard_normal((BATCH, HSS), dtype=np.float32)}
    print("smoke build only")
    _build()
    print("build ok")
